# revision 1
# baseline (speedup 1.0000x reference)
"""DistMult decoder edge-scoring kernel for Trainium2 (8 NeuronCores).

score[e] = sum_d z[src_e, d] * rel_emb[type_e, d] * z[dst_e, d]

Sharding: pure edge-parallel across 8 cores; z and rel_emb replicated.

Edges per core are bucketed by (src//25000, dst//25000) into 16 buckets so
z-row indices fit int16 against one of four z quarter-tables. Each bucket is
padded to CAP slots; slot i of a bucket lands at [i%128, i//128] in the
bucket's gathered tile (dma_gather layout). Gathers round-robin over 4 SWDGE
queues with one DMA semaphore per queue; the vector engine runs
mult/mult/reduce per bucket with ping-pong buffers. Host un-permutes scores.

Buckets are padded with -1 indices; the gather firmware trims trailing
negatives (count supplied per bucket via a gpsimd register), so padding costs
no HBM traffic. rel_emb is replicated 256x in DRAM to spread HBM row
conflicts of the hot 100-row table. If a bucket ever exceeds CAP slots the
kernel transparently recompiles with a larger capacity (safe up to ~6400
slots/bucket, far beyond anything uniform edge distributions produce).

Measured on 8 axon trn2 cores: ~0.7-1.0 ms end-to-end per call (machine-state
dependent), vs 2.8 ms for the naive per-column indirect-DMA version.
"""

import numpy as np

import concourse.bass as bass
from concourse import bacc, mybir
from concourse.bass_utils import run_bass_kernel_spmd

N_NODES = 100000
N_REL = 100
HIDDEN = 128
N_EDGES = 600000
N_CORES = 8
E_CORE = N_EDGES // N_CORES   # 75000
P = 128
NQ = 4                        # z quarter tables
ZQ = N_NODES // NQ            # 25000 rows per quarter
NB = NQ * NQ                  # 16 buckets
CAP = 5632                    # slots per bucket (mean 4687 + 14 sigma)
NSETS = 2                     # ping-pong buffer sets
RELREP = 256                  # rel_emb DRAM replication (spreads HBM row conflicts)

_cache = {}


def _build(cap=CAP, reps=1, compute=True, nsets=NSETS):
    """reps>1 repeats the full bucket sweep (for wall-clock amplification).
    compute=False drops the vector stage (pure gather throughput bench)."""
    cols = cap // P
    ci = cap // 16
    f32, i16 = mybir.dt.float32, mybir.dt.int16
    nc = bacc.Bacc("TRN2", target_bir_lowering=False, debug=False,
                   num_swdge_queues=4)

    zt = [nc.dram_tensor(f"z{q}", [ZQ, HIDDEN], f32,
                         kind="ExternalInput").ap() for q in range(NQ)]
    rel = nc.dram_tensor("rel", [RELREP * N_REL, HIDDEN], f32,
                         kind="ExternalInput").ap()
    sidx = nc.dram_tensor("sidx", [P, NB * ci], i16, kind="ExternalInput").ap()
    didx = nc.dram_tensor("didx", [P, NB * ci], i16, kind="ExternalInput").ap()
    tidx = nc.dram_tensor("tidx", [P, NB * ci], i16, kind="ExternalInput").ap()
    bcnt = nc.dram_tensor("bcnt", [1, NB], mybir.dt.int32,
                          kind="ExternalInput").ap()
    out = nc.dram_tensor("out", [P, NB * cols], f32, kind="ExternalOutput").ap()

    from contextlib import ExitStack
    with (
        nc.Block() as block,
        nc.sbuf_tensor("sidx_sb", [P, NB * ci], i16) as sidx_sb,
        nc.sbuf_tensor("didx_sb", [P, NB * ci], i16) as didx_sb,
        nc.sbuf_tensor("tidx_sb", [P, NB * ci], i16) as tidx_sb,
        nc.sbuf_tensor("bcnt_sb", [1, NB], mybir.dt.int32) as bcnt_sb,
        nc.sbuf_tensor("scores", [P, NB * cols], f32) as scores,
        nc.semaphore("io") as io,
        nc.semaphore("vdone") as vdone,
        nc.semaphore("vaux") as vaux,
        ExitStack() as stack,
    ):
        qsem = [[stack.enter_context(nc.semaphore(f"q{j}s{s}"))  # noqa: ANT232
                 for s in range(nsets)] for j in range(4)]
        gbuf = []
        for s in range(nsets):
            bufs = []
            for nm in ("src", "dst", "rel"):
                bufs.append(stack.enter_context(
                    nc.sbuf_tensor(f"{nm}g{s}", [P, cols, HIDDEN], f32)))
            gbuf.append(bufs)

        total = reps * NB

        @block.sync
        def _(sync: bass.BassEngine):
            sync.dma_start(out=sidx_sb[:], in_=sidx[:]).then_inc(io, 16)
            sync.dma_start(out=didx_sb[:], in_=didx[:]).then_inc(io, 16)
            sync.dma_start(out=tidx_sb[:], in_=tidx[:]).then_inc(io, 16)
            sync.dma_start(out=bcnt_sb[:], in_=bcnt[:]).then_inc(io, 16)
            if compute:
                sync.wait_ge(vdone, total)
            else:
                gtot = 3 * total
                for j in range(4):
                    for s_ in range(nsets):
                        n = sum(1 for g in range(gtot)
                                if g % 4 == j and (g // 3) % nsets == s_)
                        if n:
                            sync.wait_ge(qsem[j][s_], 16 * n)
            sync.dma_start(out=out[:], in_=scores[:]).then_inc(io, 16)
            sync.wait_ge(io, 64)

        @block.gpsimd
        def _(gp: bass.BassGpSimd):
            gp.wait_ge(io, 64)
            g = 0
            creg_cm = gp.register("bcnt_reg")
            creg = creg_cm.__enter__()
            for it in range(total):
                b = it % NB
                if compute and it >= nsets:
                    gp.wait_ge(vdone, it - nsets + 1)
                s_ = it % nsets
                st = gbuf[s_]
                qs, qd = b // NQ, b % NQ
                gp.reg_load(creg, bcnt_sb[0:1, b:b + 1])
                for buf, tab, isb in ((st[0], zt[qs], sidx_sb),
                                      (st[1], zt[qd], didx_sb),
                                      (st[2], rel, tidx_sb)):
                    q = g % 4
                    gp.dma_gather(
                        buf[:], tab[:], isb[:, b * ci:(b + 1) * ci],
                        cap, creg, HIDDEN,
                        single_packet=False, queue_num=q,
                    ).then_inc(qsem[q][s_], 16)
                    g += 1
            creg_cm.__exit__(None, None, None)

        @block.vector
        def _(v: bass.BassVectorEngine):
            if not compute:
                return
            cnt = [[0] * nsets for _ in range(4)]
            g = 0
            for it in range(total):
                b = it % NB
                s_ = it % nsets
                st = gbuf[s_]
                changed = set()
                for _s in range(3):
                    cnt[g % 4][s_] += 1
                    changed.add(g % 4)
                    g += 1
                for j in sorted(changed):
                    v.wait_ge(qsem[j][s_], 16 * cnt[j][s_])
                v.tensor_tensor(out=st[0][:], in0=st[0][:], in1=st[1][:],
                                op=mybir.AluOpType.mult).then_inc(vaux, 1)
                v.tensor_tensor(out=st[0][:], in0=st[0][:], in1=st[2][:],
                                op=mybir.AluOpType.mult,
                                )._wait_ge(vaux, 2 * it + 1).then_inc(vaux, 1)
                v.tensor_reduce(
                    out=scores[:, b * cols:(b + 1) * cols], in_=st[0][:],
                    axis=mybir.AxisListType.X, op=mybir.AluOpType.add,
                )._wait_ge(vaux, 2 * it + 2).then_inc(vdone, 1)

    nc.compile()
    return nc


def _wrap(idx2d):
    """[NB, CAP] int -> wrapped [128, NB*CI] int16."""
    nb, cap = idx2d.shape
    w = idx2d.reshape(nb, cap // 16, 16).transpose(0, 2, 1)  # [NB,16,CI]
    w = np.tile(w, (1, 8, 1))                                # [NB,128,CI]
    return np.concatenate(list(w), axis=1).astype(np.int16)  # [128, NB*CI]


def _prep_inputs(z, rel_emb, edge_index, edge_type, cap=CAP):
    cols = cap // P
    z = np.ascontiguousarray(z, dtype=np.float32)
    rel_emb = np.ascontiguousarray(rel_emb, dtype=np.float32)
    src = np.asarray(edge_index[0], dtype=np.int64)
    dst = np.asarray(edge_index[1], dtype=np.int64)
    typ = np.asarray(edge_type, dtype=np.int64)

    zq = [np.ascontiguousarray(z[q * ZQ:(q + 1) * ZQ]) for q in range(NQ)]
    rel_rep = np.ascontiguousarray(np.tile(rel_emb, (RELREP, 1)))

    in_maps, positions = [], []
    for c in range(N_CORES):
        sl = slice(c * E_CORE, (c + 1) * E_CORE)
        s, d, t = src[sl], dst[sl], typ[sl]
        b = (s // ZQ) * NQ + (d // ZQ)
        order = np.argsort(b, kind="stable")
        counts = np.bincount(b, minlength=NB)
        if counts.max() > cap:
            raise OverflowError(int(counts.max()))
        starts = np.zeros(NB, np.int64)
        starts[1:] = np.cumsum(counts)[:-1]
        rank = np.arange(E_CORE) - starts[b[order]]
        bo = b[order]

        sloc = np.full((NB, cap), -1, np.int64)
        dloc = np.full((NB, cap), -1, np.int64)
        tloc = np.full((NB, cap), -1, np.int64)
        sloc[bo, rank] = s[order] % ZQ
        dloc[bo, rank] = d[order] % ZQ
        tloc[bo, rank] = t[order] + N_REL * (rank % RELREP)

        # score of (bucket bb, slot r) lands at out[r%128, bb*cols + r//128]
        pos = np.empty(E_CORE, np.int64)
        pos[order] = (rank % P) * (NB * cols) + bo * cols + rank // P
        positions.append(pos)

        cnts = np.maximum(counts, 1).astype(np.int32)
        for bb in range(NB):
            if counts[bb] == 0:
                sloc[bb, 0] = 0; dloc[bb, 0] = 0; tloc[bb, 0] = 0
        in_maps.append({
            **{f"z{q}": zq[q] for q in range(NQ)},
            "rel": rel_rep,
            "bcnt": cnts.reshape(1, NB),
            "sidx": _wrap(sloc),
            "didx": _wrap(dloc),
            "tidx": _wrap(tloc),
        })
    return in_maps, positions


def kernel_run(z, rel_emb, edge_index, edge_type, trace=False):
    cap = _cache.get("cap", CAP)
    while True:
        try:
            in_maps, positions = _prep_inputs(z, rel_emb, edge_index,
                                              edge_type, cap=cap)
            break
        except OverflowError as e:
            cap = -(-int(e.args[0]) // P) * P
            _cache.pop("nc", None)
            _cache["cap"] = cap
    if "nc" not in _cache:
        _cache["nc"] = _build(cap=cap)
    nc = _cache["nc"]
    res = run_bass_kernel_spmd(nc, in_maps, core_ids=list(range(N_CORES)),
                               trace=trace)
    parts = [np.asarray(res.results[c]["out"]).reshape(-1)[positions[c]]
             for c in range(N_CORES)]
    return np.concatenate(parts).astype(np.float32, copy=False), res


def kernel(z, rel_emb, edge_index, edge_type):
    out, _ = kernel_run(z, rel_emb, edge_index, edge_type)
    return out



# revision 2
# speedup vs baseline: 1.4127x; 1.4127x over previous
"""DistMult decoder edge-scoring kernel for Trainium2 (8 NeuronCores).

score[e] = sum_d z[src_e, d] * rel_emb[type_e, d] * z[dst_e, d]

Sharding: pure edge-parallel across 8 cores; z and rel_emb replicated.

Design (v4). Per core, edges are bucketed by (src//25000, dst//25000) into 16
buckets so z-row indices fit int16 against one of four z quarter-tables, then
each bucket is processed as two half-bucket iterations of HALF=3072 slots
(24 [128,128] chunks = 6 whole PSUM banks):

 - gpsimd issues two f32 SWDGE dma_gathers per iteration (z_src, z_dst;
   512B descriptors — the sweet spot: sub-512B descriptors cost the same)
   round-robin over all 4 SWDGE queues; trailing -1 padding is trimmed via a
   per-iteration count register, so padding costs no HBM traffic. The
   measured limit is the per-queue descriptor rate (~10.8 ns/desc), so the
   rel gather is eliminated entirely (-33% descriptors):
 - rel rows are instead computed on the otherwise-idle PE as a one-hot
   matmul: stationary = host-built one-hot [100 types, 128 slots] fp8 chunk,
   moving = rel_emb [100, 128] fp16 -> psum [128 slots, 128 hidden] f32.
 - the scalar (ACT) engine copies each filled PSUM bank [128, 512] f32 to an
   SBUF fp16 rel-row tile (lossless: values are fp16 rel entries).
 - DVE runs three whole-iteration ops: mult1 zs*=zd, mult2 zs*=relrow
   (scalar_tensor_tensor), and a reduce over hidden into scores.

Four buffer sets keep the gather queues saturated (with 2 sets the queues
idle ~30% waiting on DVE). Host un-permutes scores.

Measured on 8 axon trn2 cores: ~420 us/sweep vs ~860-1010 us for the
previous 3-gather version. Max rel err ~2.4e-4 (fp16 rel rounding only).
"""

from contextlib import ExitStack

import numpy as np
import ml_dtypes

import concourse.bass as bass
from concourse import bacc, mybir
from concourse.bass_utils import run_bass_kernel_spmd

N_NODES = 100000
N_REL = 100
HIDDEN = 128
N_EDGES = 600000
N_CORES = 8
E_CORE = N_EDGES // N_CORES   # 75000
P = 128
NQ = 4                        # z quarter tables
ZQ = N_NODES // NQ            # 25000 rows per quarter
NB = NQ * NQ                  # 16 buckets
CAP = 6144                    # slots per bucket; cap % 1024 == 0 so each
                              # half-bucket is a whole number of PSUM banks
NSETS = 4                     # gather/compute buffer sets
BANKS = 8                     # PSUM banks (4 [128,128] f32 chunks each)

f32, i16 = mybir.dt.float32, mybir.dt.int16
fp16 = mybir.dt.float16
fp8 = mybir.dt.float8e4

_cache = {}


def _build(cap=CAP, reps=1, nsets=NSETS):
    cols = cap // P
    half = cap // 2
    hcols = cols // 2
    hci = half // 16              # idx columns per half-bucket
    gpb = hcols // 4              # PSUM bank fills per iteration
    nit = 2 * NB                  # iterations per sweep
    nc = bacc.Bacc("TRN2", target_bir_lowering=False, debug=False,
                   num_swdge_queues=4)

    zt = [nc.dram_tensor(f"z{q}", [ZQ, HIDDEN], f32,
                         kind="ExternalInput").ap() for q in range(NQ)]
    relt = nc.dram_tensor("relt", [N_REL, HIDDEN], fp16,
                          kind="ExternalInput").ap()
    oh = nc.dram_tensor("oh", [N_REL, NB * cap], fp8,
                        kind="ExternalInput").ap()
    sidx = nc.dram_tensor("sidx", [P, nit * hci], i16,
                          kind="ExternalInput").ap()
    didx = nc.dram_tensor("didx", [P, nit * hci], i16,
                          kind="ExternalInput").ap()
    bcnt = nc.dram_tensor("bcnt", [1, nit], mybir.dt.int32,
                          kind="ExternalInput").ap()
    out = nc.dram_tensor("out", [P, NB * cols], f32, kind="ExternalOutput").ap()

    total = reps * nit

    with (
        nc.Block() as block,
        nc.sbuf_tensor("sidx_sb", [P, nit * hci], i16) as sidx_sb,
        nc.sbuf_tensor("didx_sb", [P, nit * hci], i16) as didx_sb,
        nc.sbuf_tensor("bcnt_sb", [1, nit], mybir.dt.int32) as bcnt_sb,
        nc.sbuf_tensor("rel_sb", [N_REL, HIDDEN], fp16) as rel_sb,
        nc.sbuf_tensor("scores", [P, NB * cols], f32) as scores,
        nc.psum_tensor("ps", [P, BANKS * 4, P], f32) as ps,
        nc.semaphore("io") as io,
        nc.semaphore("psem") as psem,    # PE bank fills (global count)
        nc.semaphore("asem") as asem,    # ACT bank copies (global count)
        nc.semaphore("vaux") as vaux,    # DVE intra-iteration RAW chain
        nc.semaphore("vred") as vred,    # DVE reduce done (per iteration)
        ExitStack() as stack,
    ):
        qsem = [[stack.enter_context(nc.semaphore(f"q{j}s{s}"))  # noqa: ANT232
                 for s in range(nsets)] for j in range(4)]
        ohsem = [stack.enter_context(nc.semaphore(f"oh{s}"))
                 for s in range(nsets)]
        zsb = [stack.enter_context(nc.sbuf_tensor(f"zs{s}", [P, hcols, HIDDEN],
                                                  f32)) for s in range(nsets)]
        zdb = [stack.enter_context(nc.sbuf_tensor(f"zd{s}", [P, hcols, HIDDEN],
                                                  f32)) for s in range(nsets)]
        ohb = [stack.enter_context(nc.sbuf_tensor(f"oh{s}", [N_REL, half],
                                                  fp8)) for s in range(nsets)]
        rrb = [stack.enter_context(nc.sbuf_tensor(f"rr{s}", [P, hcols, HIDDEN],
                                                  fp16)) for s in range(nsets)]

        @block.sync
        def _(sync: bass.BassEngine):
            sync.dma_start(out=sidx_sb[:], in_=sidx[:]).then_inc(io, 16)
            sync.dma_start(out=didx_sb[:], in_=didx[:]).then_inc(io, 16)
            sync.dma_start(out=bcnt_sb[:], in_=bcnt[:]).then_inc(io, 16)
            sync.dma_start(out=rel_sb[:], in_=relt[:]).then_inc(io, 16)
            for it in range(total):
                h = it % nit
                s = it % nsets
                if it >= nsets:
                    sync.wait_ge(psem, (it - nsets + 1) * gpb)
                sync.dma_start(
                    out=ohb[s][:], in_=oh[:, h * half:(h + 1) * half],
                ).then_inc(ohsem[s], 16)
            sync.wait_ge(vred, total)
            sync.dma_start(out=out[:], in_=scores[:]).then_inc(io, 16)
            sync.wait_ge(io, 80)

        @block.gpsimd
        def _(gp: bass.BassGpSimd):
            gp.wait_ge(io, 64)
            g = 0
            creg_cm = gp.register("bcnt_reg")
            creg = creg_cm.__enter__()
            for it in range(total):
                h = it % nit
                b = h // 2
                if it >= nsets:
                    gp.wait_ge(vred, it - nsets + 1)
                s = it % nsets
                qs, qd = b // NQ, b % NQ
                gp.reg_load(creg, bcnt_sb[0:1, h:h + 1])
                for buf, tab, isb in ((zsb[s], zt[qs], sidx_sb),
                                      (zdb[s], zt[qd], didx_sb)):
                    q = g % 4
                    gp.dma_gather(
                        buf[:], tab[:], isb[:, h * hci:(h + 1) * hci],
                        half, creg, HIDDEN,
                        single_packet=False, queue_num=q,
                    ).then_inc(qsem[q][s], 16)
                    g += 1
            creg_cm.__exit__(None, None, None)

        @block.tensor
        def _(pe: bass.BassTensorEngine):
            pe.wait_ge(io, 64)
            for it in range(total):
                s = it % nsets
                pe.wait_ge(ohsem[s], 16 * (it // nsets + 1))
                for c in range(hcols):
                    C = it * hcols + c           # global chunk
                    G4 = C // 4                  # global bank-fill index
                    bank = G4 % BANKS
                    k = bank * 4 + C % 4
                    if C % 4 == 0 and G4 >= BANKS:
                        pe.wait_ge(asem, G4 - BANKS + 1)
                    mm = pe.matmul(
                        out=ps[:, k, :],
                        lhsT=ohb[s][:, c * P:(c + 1) * P],
                        rhs=rel_sb[:],
                    )
                    if C % 4 == 3:
                        mm.then_inc(psem, 1)

        @block.scalar
        def _(act: bass.BassScalarEngine):
            for it in range(total):
                s = it % nsets
                if it >= nsets:
                    act.wait_ge(vred, it - nsets + 1)
                for j in range(gpb):
                    G4 = it * gpb + j
                    act.wait_ge(psem, G4 + 1)
                    bank = G4 % BANKS
                    act.copy(
                        out=rrb[s][:, 4 * j:4 * j + 4, :],
                        in_=ps[:, bank * 4:bank * 4 + 4, :],
                    ).then_inc(asem, 1)

        @block.vector
        def _(v: bass.BassVectorEngine):
            cnt = [[0] * nsets for _ in range(4)]
            g = 0
            for it in range(total):
                h = it % nit
                b = h // 2
                hh = h % 2
                s = it % nsets
                changed = set()
                for _ in range(2):
                    cnt[g % 4][s] += 1
                    changed.add(g % 4)
                    g += 1
                for j in sorted(changed):
                    v.wait_ge(qsem[j][s], 16 * cnt[j][s])
                v.tensor_tensor(out=zsb[s][:], in0=zsb[s][:], in1=zdb[s][:],
                                op=mybir.AluOpType.mult).then_inc(vaux, 1)
                v.wait_ge(asem, (it + 1) * gpb)
                v.scalar_tensor_tensor(
                    out=zsb[s][:], in0=zsb[s][:], scalar=1.0, in1=rrb[s][:],
                    op0=mybir.AluOpType.mult, op1=mybir.AluOpType.mult,
                )._wait_ge(vaux, 2 * it + 1).then_inc(vaux, 1)
                v.tensor_reduce(
                    out=scores[:, b * cols + hh * hcols:
                               b * cols + (hh + 1) * hcols],
                    in_=zsb[s][:],
                    axis=mybir.AxisListType.X, op=mybir.AluOpType.add,
                )._wait_ge(vaux, 2 * it + 2).then_inc(vred, 1)

    nc.compile()
    return nc


def _wrap(idx2d):
    """[n, m] int -> wrapped [128, n*m//16] int16 (slot i of a row lands at
    partition i%16, column i//16; replicated to 128 partitions)."""
    n, m = idx2d.shape
    w = idx2d.reshape(n, m // 16, 16).transpose(0, 2, 1)   # [n,16,m/16]
    w = np.tile(w, (1, 8, 1))                              # [n,128,m/16]
    return np.concatenate(list(w), axis=1).astype(np.int16)


def _prep_inputs(z, rel_emb, edge_index, edge_type, cap=CAP):
    cols = cap // P
    half = cap // 2
    z = np.ascontiguousarray(z, dtype=np.float32)
    rel_emb = np.asarray(rel_emb, dtype=np.float32)
    src = np.asarray(edge_index[0], dtype=np.int64)
    dst = np.asarray(edge_index[1], dtype=np.int64)
    typ = np.asarray(edge_type, dtype=np.int64)

    zq = [np.ascontiguousarray(z[q * ZQ:(q + 1) * ZQ]) for q in range(NQ)]
    relt = rel_emb.astype(np.float16)

    in_maps, positions = [], []
    for c in range(N_CORES):
        sl = slice(c * E_CORE, (c + 1) * E_CORE)
        s, d, t = src[sl], dst[sl], typ[sl]
        b = (s // ZQ) * NQ + (d // ZQ)
        order = np.argsort(b, kind="stable")
        counts = np.bincount(b, minlength=NB)
        if counts.max() > cap:
            raise OverflowError(int(counts.max()))
        starts = np.zeros(NB, np.int64)
        starts[1:] = np.cumsum(counts)[:-1]
        rank = np.arange(E_CORE) - starts[b[order]]
        bo = b[order]

        sloc = np.full((NB, cap), -1, np.int64)
        dloc = np.full((NB, cap), -1, np.int64)
        sloc[bo, rank] = s[order] % ZQ
        dloc[bo, rank] = d[order] % ZQ
        # split into half-buckets: [2*NB, half]
        sh = np.concatenate([sloc[:, :half], sloc[:, half:]],
                            axis=1).reshape(NB, 2, half).reshape(2 * NB, half)
        dh = np.concatenate([dloc[:, :half], dloc[:, half:]],
                            axis=1).reshape(NB, 2, half).reshape(2 * NB, half)

        oh_arr = np.zeros((N_REL, NB * cap), ml_dtypes.float8_e4m3)
        oh_arr[t[order], bo * cap + rank] = 1.0

        # score of (bucket bb, slot r) lands at out[r%128, bb*cols + r//128]
        pos = np.empty(E_CORE, np.int64)
        pos[order] = (rank % P) * (NB * cols) + bo * cols + rank // P
        positions.append(pos)

        hcnt = np.empty(2 * NB, np.int64)
        hcnt[0::2] = np.minimum(counts, half)
        hcnt[1::2] = np.maximum(counts - half, 0)
        for bb in range(NB):
            if counts[bb] == 0:
                sh[2 * bb, 0] = 0; dh[2 * bb, 0] = 0
            if counts[bb] <= half:
                sh[2 * bb + 1, 0] = 0; dh[2 * bb + 1, 0] = 0
        hcnt = np.maximum(hcnt, 1).astype(np.int32)
        in_maps.append({
            **{f"z{q}": zq[q] for q in range(NQ)},
            "relt": relt,
            "oh": oh_arr,
            "bcnt": hcnt.reshape(1, 2 * NB),
            "sidx": _wrap(sh),
            "didx": _wrap(dh),
        })
    return in_maps, positions


def kernel_run(z, rel_emb, edge_index, edge_type, trace=False):
    cap = _cache.get("cap", CAP)
    while True:
        try:
            in_maps, positions = _prep_inputs(z, rel_emb, edge_index,
                                              edge_type, cap=cap)
            break
        except OverflowError as e:
            cap = -(-int(e.args[0]) // 1024) * 1024
            _cache.pop("nc", None)
            _cache["cap"] = cap
    if "nc" not in _cache:
        _cache["nc"] = _build(cap=cap)
    nc = _cache["nc"]
    res = run_bass_kernel_spmd(nc, in_maps, core_ids=list(range(N_CORES)),
                               trace=trace)
    parts = [np.asarray(res.results[c]["out"]).reshape(-1)[positions[c]]
             for c in range(N_CORES)]
    return np.concatenate(parts).astype(np.float32, copy=False), res


def kernel(z, rel_emb, edge_index, edge_type):
    out, _ = kernel_run(z, rel_emb, edge_index, edge_type)
    return out


# revision 3
# speedup vs baseline: 1.5115x; 1.0700x over previous
"""DistMult decoder edge-scoring kernel for Trainium2 (8 NeuronCores).

score[e] = sum_d z[src_e, d] * rel_emb[type_e, d] * z[dst_e, d]

Sharding: pure edge-parallel across 8 cores; z and rel_emb replicated.

Design (v4). Per core, edges are bucketed by (src//25000, dst//25000) into 16
buckets so z-row indices fit int16 against one of four z quarter-tables, then
each bucket is processed as two half-bucket iterations of HALF=3072 slots
(24 [128,128] chunks = 6 whole PSUM banks):

 - gpsimd issues two f32 SWDGE dma_gathers per iteration (z_src, z_dst;
   512B descriptors — the sweet spot: sub-512B descriptors cost the same)
   round-robin over all 4 SWDGE queues; trailing -1 padding is trimmed via a
   per-iteration count register, so padding costs no HBM traffic. The
   measured limit is the per-queue descriptor rate (~10.8 ns/desc), so the
   rel gather is eliminated entirely (-33% descriptors):
 - rel rows are instead computed on the otherwise-idle PE as a one-hot
   matmul: stationary = host-built one-hot [100 types, 128 slots] fp8 chunk,
   moving = rel_emb [100, 128] fp16 -> psum [128 slots, 128 hidden] f32.
 - the scalar (ACT) engine copies each filled PSUM bank [128, 512] f32 to an
   SBUF fp16 rel-row tile (lossless: values are fp16 rel entries).
 - DVE runs three whole-iteration ops: mult1 zs*=zd, mult2 zs*=relrow
   (scalar_tensor_tensor), and a reduce over hidden into scores.

Four buffer sets keep the gather queues saturated (with 2 sets the queues
idle ~30% waiting on DVE). Host un-permutes scores.

Measured on 8 axon trn2 cores: ~420 us/sweep vs ~860-1010 us for the
previous 3-gather version. Max rel err ~2.4e-4 (fp16 rel rounding only).
"""

from contextlib import ExitStack

import numpy as np
import ml_dtypes

import concourse.bass as bass
from concourse import bacc, mybir
from concourse.bass_utils import run_bass_kernel_spmd

N_NODES = 100000
N_REL = 100
HIDDEN = 128
N_EDGES = 600000
N_CORES = 8
E_CORE = N_EDGES // N_CORES   # 75000
P = 128
NQ = 4                        # z quarter tables
ZQ = N_NODES // NQ            # 25000 rows per quarter
NB = NQ * NQ                  # 16 buckets
CAP = 6144                    # slots per bucket; cap % 1024 == 0 so each
                              # half-bucket is a whole number of PSUM banks
NSETS = 4                     # gather/compute buffer sets
BANKS = 8                     # PSUM banks (4 [128,128] f32 chunks each)

f32, i16 = mybir.dt.float32, mybir.dt.int16
fp16 = mybir.dt.float16
fp8 = mybir.dt.float8e4

_cache = {}


def _build(cap=CAP, reps=1, nsets=NSETS):
    cols = cap // P
    half = cap // 2
    hcols = cols // 2
    hci = half // 16              # idx columns per half-bucket
    gpb = hcols // 4              # PSUM bank fills per iteration
    nit = 2 * NB                  # iterations per sweep
    nc = bacc.Bacc("TRN2", target_bir_lowering=False, debug=False,
                   num_swdge_queues=4)

    zt = [nc.dram_tensor(f"z{q}", [ZQ, HIDDEN], f32,
                         kind="ExternalInput").ap() for q in range(NQ)]
    relt = nc.dram_tensor("relt", [N_REL, HIDDEN], fp16,
                          kind="ExternalInput").ap()
    oh = nc.dram_tensor("oh", [N_REL, NB * cap], fp8,
                        kind="ExternalInput").ap()
    sidx = nc.dram_tensor("sidx", [P, nit * hci], i16,
                          kind="ExternalInput").ap()
    didx = nc.dram_tensor("didx", [P, nit * hci], i16,
                          kind="ExternalInput").ap()
    bcnt = nc.dram_tensor("bcnt", [1, nit], mybir.dt.int32,
                          kind="ExternalInput").ap()
    out = nc.dram_tensor("out", [P, NB * cols], f32, kind="ExternalOutput").ap()

    total = reps * nit

    with (
        nc.Block() as block,
        nc.sbuf_tensor("sidx_sb", [P, nit * hci], i16) as sidx_sb,
        nc.sbuf_tensor("didx_sb", [P, nit * hci], i16) as didx_sb,
        nc.sbuf_tensor("bcnt_sb", [1, nit], mybir.dt.int32) as bcnt_sb,
        nc.sbuf_tensor("rel_sb", [N_REL, HIDDEN], fp16) as rel_sb,
        nc.sbuf_tensor("scores", [P, NB * cols], f32) as scores,
        nc.psum_tensor("ps", [P, BANKS * 4, P], f32) as ps,
        nc.semaphore("io") as io,
        nc.semaphore("psem") as psem,    # PE bank fills (global count)
        nc.semaphore("asem") as asem,    # ACT bank copies (global count)
        nc.semaphore("vaux") as vaux,    # DVE intra-iteration RAW chain
        nc.semaphore("vred") as vred,    # DVE reduce done (per iteration)
        ExitStack() as stack,
    ):
        qsem = [[stack.enter_context(nc.semaphore(f"q{j}s{s}"))  # noqa: ANT232
                 for s in range(nsets)] for j in range(4)]
        ohsem = [stack.enter_context(nc.semaphore(f"oh{s}"))
                 for s in range(nsets)]
        zsb = [stack.enter_context(nc.sbuf_tensor(f"zs{s}", [P, hcols, HIDDEN],
                                                  f32)) for s in range(nsets)]
        zdb = [stack.enter_context(nc.sbuf_tensor(f"zd{s}", [P, hcols, HIDDEN],
                                                  f32)) for s in range(nsets)]
        ohb = [stack.enter_context(nc.sbuf_tensor(f"oh{s}", [N_REL, half],
                                                  fp8)) for s in range(nsets)]
        rrb = [stack.enter_context(nc.sbuf_tensor(f"rr{s}", [P, hcols, HIDDEN],
                                                  fp16)) for s in range(nsets)]

        @block.sync
        def _(sync: bass.BassEngine):
            sync.dma_start(out=sidx_sb[:], in_=sidx[:]).then_inc(io, 16)
            sync.dma_start(out=didx_sb[:], in_=didx[:]).then_inc(io, 16)
            sync.dma_start(out=bcnt_sb[:], in_=bcnt[:]).then_inc(io, 16)
            sync.dma_start(out=rel_sb[:], in_=relt[:]).then_inc(io, 16)
            for it in range(total):
                h = it % nit
                s = it % nsets
                if it >= nsets:
                    sync.wait_ge(psem, (it - nsets + 1) * gpb)
                sync.dma_start(
                    out=ohb[s][:], in_=oh[:, h * half:(h + 1) * half],
                ).then_inc(ohsem[s], 16)
            sync.wait_ge(vred, total)
            sync.dma_start(out=out[:], in_=scores[:]).then_inc(io, 16)
            sync.wait_ge(io, 80)

        @block.gpsimd
        def _(gp: bass.BassGpSimd):
            gp.wait_ge(io, 64)
            g = 0
            creg_cm = gp.register("bcnt_reg")
            creg = creg_cm.__enter__()
            for it in range(total):
                h = it % nit
                b = h // 2
                if it >= nsets:
                    gp.wait_ge(vred, it - nsets + 1)
                s = it % nsets
                qs, qd = b // NQ, b % NQ
                gp.reg_load(creg, bcnt_sb[0:1, h:h + 1])
                for buf, tab, isb in ((zsb[s], zt[qs], sidx_sb),
                                      (zdb[s], zt[qd], didx_sb)):
                    # rotate by iteration so each queue gets a mix of full
                    # half-A and short half-B gathers (g%4 alone sends all
                    # big gathers to queues 0/1 -> 65/35 desc imbalance)
                    q = (g + it) % 4
                    gp.dma_gather(
                        buf[:], tab[:], isb[:, h * hci:(h + 1) * hci],
                        half, creg, HIDDEN,
                        single_packet=False, queue_num=q,
                    ).then_inc(qsem[q][s], 16)
                    g += 1
            creg_cm.__exit__(None, None, None)

        @block.tensor
        def _(pe: bass.BassTensorEngine):
            pe.wait_ge(io, 64)
            for it in range(total):
                s = it % nsets
                pe.wait_ge(ohsem[s], 16 * (it // nsets + 1))
                for c in range(hcols):
                    C = it * hcols + c           # global chunk
                    G4 = C // 4                  # global bank-fill index
                    bank = G4 % BANKS
                    k = bank * 4 + C % 4
                    if C % 4 == 0 and G4 >= BANKS:
                        pe.wait_ge(asem, G4 - BANKS + 1)
                    mm = pe.matmul(
                        out=ps[:, k, :],
                        lhsT=ohb[s][:, c * P:(c + 1) * P],
                        rhs=rel_sb[:],
                    )
                    if C % 4 == 3:
                        mm.then_inc(psem, 1)

        @block.scalar
        def _(act: bass.BassScalarEngine):
            for it in range(total):
                s = it % nsets
                if it >= nsets:
                    act.wait_ge(vred, it - nsets + 1)
                for j in range(gpb):
                    G4 = it * gpb + j
                    act.wait_ge(psem, G4 + 1)
                    bank = G4 % BANKS
                    act.copy(
                        out=rrb[s][:, 4 * j:4 * j + 4, :],
                        in_=ps[:, bank * 4:bank * 4 + 4, :],
                    ).then_inc(asem, 1)

        @block.vector
        def _(v: bass.BassVectorEngine):
            cnt = [[0] * nsets for _ in range(4)]
            g = 0
            for it in range(total):
                h = it % nit
                b = h // 2
                hh = h % 2
                s = it % nsets
                changed = set()
                for _ in range(2):
                    q = (g + it) % 4
                    cnt[q][s] += 1
                    changed.add(q)
                    g += 1
                for j in sorted(changed):
                    v.wait_ge(qsem[j][s], 16 * cnt[j][s])
                v.tensor_tensor(out=zsb[s][:], in0=zsb[s][:], in1=zdb[s][:],
                                op=mybir.AluOpType.mult).then_inc(vaux, 1)
                v.wait_ge(asem, (it + 1) * gpb)
                v.scalar_tensor_tensor(
                    out=zsb[s][:], in0=zsb[s][:], scalar=1.0, in1=rrb[s][:],
                    op0=mybir.AluOpType.mult, op1=mybir.AluOpType.mult,
                )._wait_ge(vaux, 2 * it + 1).then_inc(vaux, 1)
                v.tensor_reduce(
                    out=scores[:, b * cols + hh * hcols:
                               b * cols + (hh + 1) * hcols],
                    in_=zsb[s][:],
                    axis=mybir.AxisListType.X, op=mybir.AluOpType.add,
                )._wait_ge(vaux, 2 * it + 2).then_inc(vred, 1)

    nc.compile()
    return nc


def _wrap(idx2d):
    """[n, m] int -> wrapped [128, n*m//16] int16 (slot i of a row lands at
    partition i%16, column i//16; replicated to 128 partitions)."""
    n, m = idx2d.shape
    w = idx2d.reshape(n, m // 16, 16).transpose(0, 2, 1)   # [n,16,m/16]
    w = np.tile(w, (1, 8, 1))                              # [n,128,m/16]
    return np.concatenate(list(w), axis=1).astype(np.int16)


def _prep_inputs(z, rel_emb, edge_index, edge_type, cap=CAP):
    cols = cap // P
    half = cap // 2
    z = np.ascontiguousarray(z, dtype=np.float32)
    rel_emb = np.asarray(rel_emb, dtype=np.float32)
    src = np.asarray(edge_index[0], dtype=np.int64)
    dst = np.asarray(edge_index[1], dtype=np.int64)
    typ = np.asarray(edge_type, dtype=np.int64)

    zq = [np.ascontiguousarray(z[q * ZQ:(q + 1) * ZQ]) for q in range(NQ)]
    relt = rel_emb.astype(np.float16)

    in_maps, positions = [], []
    for c in range(N_CORES):
        sl = slice(c * E_CORE, (c + 1) * E_CORE)
        s, d, t = src[sl], dst[sl], typ[sl]
        b = (s // ZQ) * NQ + (d // ZQ)
        order = np.argsort(b, kind="stable")
        counts = np.bincount(b, minlength=NB)
        if counts.max() > cap:
            raise OverflowError(int(counts.max()))
        starts = np.zeros(NB, np.int64)
        starts[1:] = np.cumsum(counts)[:-1]
        rank = np.arange(E_CORE) - starts[b[order]]
        bo = b[order]

        sloc = np.full((NB, cap), -1, np.int64)
        dloc = np.full((NB, cap), -1, np.int64)
        sloc[bo, rank] = s[order] % ZQ
        dloc[bo, rank] = d[order] % ZQ
        # split into half-buckets: [2*NB, half]
        sh = np.concatenate([sloc[:, :half], sloc[:, half:]],
                            axis=1).reshape(NB, 2, half).reshape(2 * NB, half)
        dh = np.concatenate([dloc[:, :half], dloc[:, half:]],
                            axis=1).reshape(NB, 2, half).reshape(2 * NB, half)

        oh_arr = np.zeros((N_REL, NB * cap), ml_dtypes.float8_e4m3)
        oh_arr[t[order], bo * cap + rank] = 1.0

        # score of (bucket bb, slot r) lands at out[r%128, bb*cols + r//128]
        pos = np.empty(E_CORE, np.int64)
        pos[order] = (rank % P) * (NB * cols) + bo * cols + rank // P
        positions.append(pos)

        hcnt = np.empty(2 * NB, np.int64)
        hcnt[0::2] = np.minimum(counts, half)
        hcnt[1::2] = np.maximum(counts - half, 0)
        for bb in range(NB):
            if counts[bb] == 0:
                sh[2 * bb, 0] = 0; dh[2 * bb, 0] = 0
            if counts[bb] <= half:
                sh[2 * bb + 1, 0] = 0; dh[2 * bb + 1, 0] = 0
        hcnt = np.maximum(hcnt, 1).astype(np.int32)
        in_maps.append({
            **{f"z{q}": zq[q] for q in range(NQ)},
            "relt": relt,
            "oh": oh_arr,
            "bcnt": hcnt.reshape(1, 2 * NB),
            "sidx": _wrap(sh),
            "didx": _wrap(dh),
        })
    return in_maps, positions


def kernel_run(z, rel_emb, edge_index, edge_type, trace=False):
    cap = _cache.get("cap", CAP)
    while True:
        try:
            in_maps, positions = _prep_inputs(z, rel_emb, edge_index,
                                              edge_type, cap=cap)
            break
        except OverflowError as e:
            cap = -(-int(e.args[0]) // 1024) * 1024
            _cache.pop("nc", None)
            _cache["cap"] = cap
    if "nc" not in _cache:
        _cache["nc"] = _build(cap=cap)
    nc = _cache["nc"]
    res = run_bass_kernel_spmd(nc, in_maps, core_ids=list(range(N_CORES)),
                               trace=trace)
    parts = [np.asarray(res.results[c]["out"]).reshape(-1)[positions[c]]
             for c in range(N_CORES)]
    return np.concatenate(parts).astype(np.float32, copy=False), res


def kernel(z, rel_emb, edge_index, edge_type):
    out, _ = kernel_run(z, rel_emb, edge_index, edge_type)
    return out


# revision 4
# speedup vs baseline: 1.5311x; 1.0130x over previous
"""DistMult decoder edge-scoring kernel for Trainium2 (8 NeuronCores).

score[e] = sum_d z[src_e, d] * rel_emb[type_e, d] * z[dst_e, d]

Sharding: pure edge-parallel across 8 cores; z and rel_emb replicated.

Design (v4). Per core, edges are bucketed by (src//25000, dst//25000) into 16
buckets so z-row indices fit int16 against one of four z quarter-tables, then
each bucket is processed as two half-bucket iterations of HALF=3072 slots
(24 [128,128] chunks = 6 whole PSUM banks):

 - gpsimd issues two f32 SWDGE dma_gathers per iteration (z_src, z_dst;
   512B descriptors — the sweet spot: sub-512B descriptors cost the same)
   round-robin over all 4 SWDGE queues; trailing -1 padding is trimmed via a
   per-iteration count register, so padding costs no HBM traffic. The
   measured limit is the per-queue descriptor rate (~10.8 ns/desc), so the
   rel gather is eliminated entirely (-33% descriptors):
 - rel rows are instead computed on the otherwise-idle PE as a one-hot
   matmul: stationary = host-built one-hot [100 types, 128 slots] fp8 chunk,
   moving = rel_emb [100, 128] fp16 -> psum [128 slots, 128 hidden] f32.
 - the scalar (ACT) engine copies each filled PSUM bank [128, 512] f32 to an
   SBUF fp16 rel-row tile (lossless: values are fp16 rel entries).
 - DVE runs three whole-iteration ops: mult1 zs*=zd, mult2 zs*=relrow
   (scalar_tensor_tensor), and a reduce over hidden into scores.

Four buffer sets keep the gather queues saturated (with 2 sets the queues
idle ~30% waiting on DVE). Host un-permutes scores.

Measured on 8 axon trn2 cores: ~420 us/sweep vs ~860-1010 us for the
previous 3-gather version. Max rel err ~2.4e-4 (fp16 rel rounding only).
"""

from contextlib import ExitStack

import numpy as np
import ml_dtypes

import concourse.bass as bass
from concourse import bacc, mybir
from concourse.bass_utils import run_bass_kernel_spmd

N_NODES = 100000
N_REL = 100
HIDDEN = 128
N_EDGES = 600000
N_CORES = 8
E_CORE = N_EDGES // N_CORES   # 75000
P = 128
NQ = 4                        # z quarter tables
ZQ = N_NODES // NQ            # 25000 rows per quarter
NB = NQ * NQ                  # 16 buckets
CAP = 6144                    # slots per bucket; cap % 1024 == 0 so each
                              # half-bucket is a whole number of PSUM banks
NSETS = 5                     # gather/compute buffer sets (5 fits: ~197KB/partition)
BANKS = 8                     # PSUM banks (4 [128,128] f32 chunks each)

f32, i16 = mybir.dt.float32, mybir.dt.int16
fp16 = mybir.dt.float16
fp8 = mybir.dt.float8e4

_cache = {}


def _build(cap=CAP, reps=1, nsets=NSETS):
    cols = cap // P
    half = cap // 2
    hcols = cols // 2
    hci = half // 16              # idx columns per half-bucket
    gpb = hcols // 4              # PSUM bank fills per iteration
    nit = 2 * NB                  # iterations per sweep
    nc = bacc.Bacc("TRN2", target_bir_lowering=False, debug=False,
                   num_swdge_queues=4)

    zt = [nc.dram_tensor(f"z{q}", [ZQ, HIDDEN], f32,
                         kind="ExternalInput").ap() for q in range(NQ)]
    relt = nc.dram_tensor("relt", [N_REL, HIDDEN], fp16,
                          kind="ExternalInput").ap()
    oh = nc.dram_tensor("oh", [N_REL, NB * cap], fp8,
                        kind="ExternalInput").ap()
    sidx = nc.dram_tensor("sidx", [P, nit * hci], i16,
                          kind="ExternalInput").ap()
    didx = nc.dram_tensor("didx", [P, nit * hci], i16,
                          kind="ExternalInput").ap()
    bcnt = nc.dram_tensor("bcnt", [1, nit], mybir.dt.int32,
                          kind="ExternalInput").ap()
    out = nc.dram_tensor("out", [P, NB * cols], f32, kind="ExternalOutput").ap()

    total = reps * nit

    with (
        nc.Block() as block,
        nc.sbuf_tensor("sidx_sb", [P, nit * hci], i16) as sidx_sb,
        nc.sbuf_tensor("didx_sb", [P, nit * hci], i16) as didx_sb,
        nc.sbuf_tensor("bcnt_sb", [1, nit], mybir.dt.int32) as bcnt_sb,
        nc.sbuf_tensor("rel_sb", [N_REL, HIDDEN], fp16) as rel_sb,
        nc.sbuf_tensor("scores", [P, NB * cols], f32) as scores,
        nc.psum_tensor("ps", [P, BANKS * 4, P], f32) as ps,
        nc.semaphore("io") as io,
        nc.semaphore("psem") as psem,    # PE bank fills (global count)
        nc.semaphore("asem") as asem,    # ACT bank copies (global count)
        nc.semaphore("vaux") as vaux,    # DVE intra-iteration RAW chain
        nc.semaphore("vred") as vred,    # DVE reduce done (per iteration)
        ExitStack() as stack,
    ):
        qsem = [[stack.enter_context(nc.semaphore(f"q{j}s{s}"))  # noqa: ANT232
                 for s in range(nsets)] for j in range(4)]
        ohsem = [stack.enter_context(nc.semaphore(f"oh{s}"))
                 for s in range(nsets)]
        zsb = [stack.enter_context(nc.sbuf_tensor(f"zs{s}", [P, hcols, HIDDEN],
                                                  f32)) for s in range(nsets)]
        zdb = [stack.enter_context(nc.sbuf_tensor(f"zd{s}", [P, hcols, HIDDEN],
                                                  f32)) for s in range(nsets)]
        ohb = [stack.enter_context(nc.sbuf_tensor(f"oh{s}", [N_REL, half],
                                                  fp8)) for s in range(nsets)]
        rrb = [stack.enter_context(nc.sbuf_tensor(f"rr{s}", [P, hcols, HIDDEN],
                                                  fp16)) for s in range(nsets)]

        @block.sync
        def _(sync: bass.BassEngine):
            sync.dma_start(out=sidx_sb[:], in_=sidx[:]).then_inc(io, 16)
            sync.dma_start(out=didx_sb[:], in_=didx[:]).then_inc(io, 16)
            sync.dma_start(out=bcnt_sb[:], in_=bcnt[:]).then_inc(io, 16)
            sync.dma_start(out=rel_sb[:], in_=relt[:]).then_inc(io, 16)
            for it in range(total):
                h = it % nit
                s = it % nsets
                if it >= nsets:
                    sync.wait_ge(psem, (it - nsets + 1) * gpb)
                sync.dma_start(
                    out=ohb[s][:], in_=oh[:, h * half:(h + 1) * half],
                ).then_inc(ohsem[s], 16)
            sync.wait_ge(vred, total)
            sync.dma_start(out=out[:], in_=scores[:]).then_inc(io, 16)
            sync.wait_ge(io, 80)

        @block.gpsimd
        def _(gp: bass.BassGpSimd):
            gp.wait_ge(io, 64)
            g = 0
            creg_cm = gp.register("bcnt_reg")
            creg = creg_cm.__enter__()
            for it in range(total):
                h = it % nit
                b = h // 2
                if it >= nsets:
                    gp.wait_ge(vred, it - nsets + 1)
                s = it % nsets
                qs, qd = b // NQ, b % NQ
                gp.reg_load(creg, bcnt_sb[0:1, h:h + 1])
                for buf, tab, isb in ((zsb[s], zt[qs], sidx_sb),
                                      (zdb[s], zt[qd], didx_sb)):
                    # rotate by iteration so each queue gets a mix of full
                    # half-A and short half-B gathers (g%4 alone sends all
                    # big gathers to queues 0/1 -> 65/35 desc imbalance)
                    q = (g + it) % 4
                    gp.dma_gather(
                        buf[:], tab[:], isb[:, h * hci:(h + 1) * hci],
                        half, creg, HIDDEN,
                        single_packet=False, queue_num=q,
                    ).then_inc(qsem[q][s], 16)
                    g += 1
            creg_cm.__exit__(None, None, None)

        @block.tensor
        def _(pe: bass.BassTensorEngine):
            pe.wait_ge(io, 64)
            for it in range(total):
                s = it % nsets
                pe.wait_ge(ohsem[s], 16 * (it // nsets + 1))
                for c in range(hcols):
                    C = it * hcols + c           # global chunk
                    G4 = C // 4                  # global bank-fill index
                    bank = G4 % BANKS
                    k = bank * 4 + C % 4
                    if C % 4 == 0 and G4 >= BANKS:
                        pe.wait_ge(asem, G4 - BANKS + 1)
                    mm = pe.matmul(
                        out=ps[:, k, :],
                        lhsT=ohb[s][:, c * P:(c + 1) * P],
                        rhs=rel_sb[:],
                    )
                    if C % 4 == 3:
                        mm.then_inc(psem, 1)

        @block.scalar
        def _(act: bass.BassScalarEngine):
            for it in range(total):
                s = it % nsets
                if it >= nsets:
                    act.wait_ge(vred, it - nsets + 1)
                for j in range(gpb):
                    G4 = it * gpb + j
                    act.wait_ge(psem, G4 + 1)
                    bank = G4 % BANKS
                    act.copy(
                        out=rrb[s][:, 4 * j:4 * j + 4, :],
                        in_=ps[:, bank * 4:bank * 4 + 4, :],
                    ).then_inc(asem, 1)

        @block.vector
        def _(v: bass.BassVectorEngine):
            cnt = [[0] * nsets for _ in range(4)]
            g = 0
            for it in range(total):
                h = it % nit
                b = h // 2
                hh = h % 2
                s = it % nsets
                changed = set()
                for _ in range(2):
                    q = (g + it) % 4
                    cnt[q][s] += 1
                    changed.add(q)
                    g += 1
                for j in sorted(changed):
                    v.wait_ge(qsem[j][s], 16 * cnt[j][s])
                v.tensor_tensor(out=zsb[s][:], in0=zsb[s][:], in1=zdb[s][:],
                                op=mybir.AluOpType.mult).then_inc(vaux, 1)
                v.wait_ge(asem, (it + 1) * gpb)
                v.scalar_tensor_tensor(
                    out=zsb[s][:], in0=zsb[s][:], scalar=1.0, in1=rrb[s][:],
                    op0=mybir.AluOpType.mult, op1=mybir.AluOpType.mult,
                )._wait_ge(vaux, 2 * it + 1).then_inc(vaux, 1)
                v.tensor_reduce(
                    out=scores[:, b * cols + hh * hcols:
                               b * cols + (hh + 1) * hcols],
                    in_=zsb[s][:],
                    axis=mybir.AxisListType.X, op=mybir.AluOpType.add,
                )._wait_ge(vaux, 2 * it + 2).then_inc(vred, 1)

    nc.compile()
    return nc


def _wrap(idx2d):
    """[n, m] int -> wrapped [128, n*m//16] int16 (slot i of a row lands at
    partition i%16, column i//16; replicated to 128 partitions)."""
    n, m = idx2d.shape
    w = idx2d.reshape(n, m // 16, 16).transpose(0, 2, 1)   # [n,16,m/16]
    w = np.tile(w, (1, 8, 1))                              # [n,128,m/16]
    return np.concatenate(list(w), axis=1).astype(np.int16)


def _prep_inputs(z, rel_emb, edge_index, edge_type, cap=CAP):
    cols = cap // P
    half = cap // 2
    z = np.ascontiguousarray(z, dtype=np.float32)
    rel_emb = np.asarray(rel_emb, dtype=np.float32)
    src = np.asarray(edge_index[0], dtype=np.int64)
    dst = np.asarray(edge_index[1], dtype=np.int64)
    typ = np.asarray(edge_type, dtype=np.int64)

    zq = [np.ascontiguousarray(z[q * ZQ:(q + 1) * ZQ]) for q in range(NQ)]
    relt = rel_emb.astype(np.float16)

    in_maps, positions = [], []
    for c in range(N_CORES):
        sl = slice(c * E_CORE, (c + 1) * E_CORE)
        s, d, t = src[sl], dst[sl], typ[sl]
        b = (s // ZQ) * NQ + (d // ZQ)
        order = np.argsort(b, kind="stable")
        counts = np.bincount(b, minlength=NB)
        if counts.max() > cap:
            raise OverflowError(int(counts.max()))
        starts = np.zeros(NB, np.int64)
        starts[1:] = np.cumsum(counts)[:-1]
        rank = np.arange(E_CORE) - starts[b[order]]
        bo = b[order]

        sloc = np.full((NB, cap), -1, np.int64)
        dloc = np.full((NB, cap), -1, np.int64)
        sloc[bo, rank] = s[order] % ZQ
        dloc[bo, rank] = d[order] % ZQ
        # split into half-buckets: [2*NB, half]
        sh = np.concatenate([sloc[:, :half], sloc[:, half:]],
                            axis=1).reshape(NB, 2, half).reshape(2 * NB, half)
        dh = np.concatenate([dloc[:, :half], dloc[:, half:]],
                            axis=1).reshape(NB, 2, half).reshape(2 * NB, half)

        oh_arr = np.zeros((N_REL, NB * cap), ml_dtypes.float8_e4m3)
        oh_arr[t[order], bo * cap + rank] = 1.0

        # score of (bucket bb, slot r) lands at out[r%128, bb*cols + r//128]
        pos = np.empty(E_CORE, np.int64)
        pos[order] = (rank % P) * (NB * cols) + bo * cols + rank // P
        positions.append(pos)

        hcnt = np.empty(2 * NB, np.int64)
        hcnt[0::2] = np.minimum(counts, half)
        hcnt[1::2] = np.maximum(counts - half, 0)
        for bb in range(NB):
            if counts[bb] == 0:
                sh[2 * bb, 0] = 0; dh[2 * bb, 0] = 0
            if counts[bb] <= half:
                sh[2 * bb + 1, 0] = 0; dh[2 * bb + 1, 0] = 0
        hcnt = np.maximum(hcnt, 1).astype(np.int32)
        in_maps.append({
            **{f"z{q}": zq[q] for q in range(NQ)},
            "relt": relt,
            "oh": oh_arr,
            "bcnt": hcnt.reshape(1, 2 * NB),
            "sidx": _wrap(sh),
            "didx": _wrap(dh),
        })
    return in_maps, positions


def kernel_run(z, rel_emb, edge_index, edge_type, trace=False):
    cap = _cache.get("cap", CAP)
    while True:
        try:
            in_maps, positions = _prep_inputs(z, rel_emb, edge_index,
                                              edge_type, cap=cap)
            break
        except OverflowError as e:
            cap = -(-int(e.args[0]) // 1024) * 1024
            _cache.pop("nc", None)
            _cache["cap"] = cap
    if "nc" not in _cache:
        _cache["nc"] = _build(cap=cap)
    nc = _cache["nc"]
    res = run_bass_kernel_spmd(nc, in_maps, core_ids=list(range(N_CORES)),
                               trace=trace)
    parts = [np.asarray(res.results[c]["out"]).reshape(-1)[positions[c]]
             for c in range(N_CORES)]
    return np.concatenate(parts).astype(np.float32, copy=False), res


def kernel(z, rel_emb, edge_index, edge_type):
    out, _ = kernel_run(z, rel_emb, edge_index, edge_type)
    return out
